# revision 78
# baseline (speedup 1.0000x reference)
"""Involution kernel for Trainium2, 8-core data-parallel (1 batch image per core).

Reference computation (per image, NHWC, C=64, G=4 groups, K=3, reduction 4):
    t    = relu(BN(x @ w1 + b1))            # [H,W,16]
    kern = t @ w2 + b2                      # [H,W,36], e = (ki*3+kj)*4 + g
    out[h,w,c] = sum_p kern[h,w, 4p + c%4] * xpad[h+di, w+dj, c]

Device strategy (v3, ~119us/core in the CoreSim cost model vs 354us for v1):
  * 256 subtiles of 12x12 interior; each lives in one SBUF partition's free
    dim with a 1-ring halo (14x14) for x.  3x3 shifts are free-dim offsets.
  * kern is computed ONLY at the 144 interior positions (v1 computed 196).
  * PE does both 1x1 convs.  mm1: lhsT=w1a[64,17], rhs=xt[64,512] tiles
    (xt = x transposed on host, block-local (q,st) pixel order).  BN is
    folded into w1a; the BN/b1 bias folds into the PSUM->SBUF Relu
    evacuation on the Activation engine (per-partition bias).  w1a column
    16 is zero with bias 1 so tp row 16 == 1; w2b row 16 then adds b2
    during mm2 (lhsT=tp[:,q*128:+128], rhs=[w2;b2][17,36]).
  * The 9-tap multiply-accumulate is column-split between DVE (bf16 2x
    mode, c below cd) and the GPSIMD/Pool engine (c above cd), writing
    disjoint channel ranges of shared acc/prod tiles; cd alternates 40/36
    per chunk to balance the engines at ~105us each.
  * Latency: demand-ordered input DMA slices, mm2 chunks emitted as tp
    columns complete (keeps Act's in-order queue from parking kern evacs),
    block 1 streamed in descending q to match its big-chunk-first taps,
    block 1 inputs issued before block 0's last chunk, PE pstate warmup,
    Act table preload, consts packed into one DMA (b1 as f32 bit-pairs).
  * All bulk data bf16; host pre-builds the two layouts.
"""

import numpy as np
import ml_dtypes

import concourse.bass as bass
import concourse.bacc as bacc
import concourse.mybir as mybir
from concourse.tile import TileContext
from concourse.bass_utils import run_bass_kernel_spmd

BF16 = mybir.dt.bfloat16
F32 = mybir.dt.float32
NPF32 = np.float32
NPBF16 = ml_dtypes.bfloat16

B, H, W, C = 8, 192, 192, 64
G, K, CR, E = 4, 3, 16, 36
BN_EPS = 1e-3
S = 12                 # subtile interior
S1 = S + 2             # 1-ring padded subtile size (14)
NG = H // S            # 16 subtiles per axis
NST = NG * NG          # 256 subtiles
NQ = S * S             # 144 interior positions per subtile
F1 = S1 * S1 * C       # 12544 x free elems per subtile (1-ring)
NB = 2                 # partition blocks of 128 subtiles
NPIX_BLK = 128 * NQ    # 18432 pixel columns per block
XT_Q = NPIX_BLK // 4   # xt streamed in quarters (4608 cols)
MM1_PS = 1024          # mm1 psum tile cols (2 banks)
MM1_MM = 512           # cols per mm1 matmul
KCH = 12               # q positions per mm2 psum tile (aligned to qh rows)
# xt stream pieces (col0, ncols): small head pieces aligned to 12-q
# (1536-col) boundaries so the first tap chunks' kern evacs clear fast.
# mm2 chunks are emitted demand-driven as tp columns complete, so Act's
# serial stream never parks kern evacs behind not-yet-loaded xt pieces.
# x2 arrives in row-slices sized to each tap chunk's halo rows.
XT_PIECES0 = ((0, 1536), (1536, 3072), (4608, 4608), (9216, 4608),
              (13824, 4608))
X2ROWS0 = {0: (0, 3), 1: (3, 6), 2: (6, 8), 4: (8, S1)}
# block 1 consumes taps big-chunk-first (q 72:144 first), so its xt
# pieces stream in descending column order and kern chunks are emitted
# in descending q order; x2 rows 6:14 (needed by the 72q chunk) first
XT_PIECES1 = ((13824, 4608), (9216, 4608), (4608, 4608), (1536, 3072),
              (0, 1536))
X2ROWS1 = {0: (6, S1), 3: (0, 6)}
# tap/acc chunks: (q0, qch, cd) — cd alternates 40/36 to balance DVE/Pool;
# block 0 runs small-first (data arrives incrementally), block 1 runs
# big-first (shorter drain tail)
CHUNKS0 = ((0, 12, 40), (12, 24, 36), (36, 36, 36), (72, 72, 40))
CHUNKS1 = ((72, 72, 40), (36, 36, 36), (12, 24, 36), (0, 12, 40))

_CACHE = {}


def _build_program():
    if "nc" in _CACHE:
        return _CACHE["nc"]
    nc = bacc.Bacc(None, target_bir_lowering=False)
    CW = CR + E + 4  # 56 packed const cols riding ahead of the xt stream
    x2_d = nc.dram_tensor("x2", [NST, F1], BF16, kind="ExternalInput")
    # xt cols 0:56 carry the packed consts (w1a | w2b | b1a-as-f32-bit-
    # pairs) so no separate const DMA occupies the critical head window
    xt_d = nc.dram_tensor("xt", [C, CW + NB * NPIX_BLK], BF16,
                          kind="ExternalInput")
    o_d = nc.dram_tensor("o", [NST, NQ * C], BF16, kind="ExternalOutput")

    relu = mybir.ActivationFunctionType.Relu
    mult = mybir.AluOpType.mult
    add = mybir.AluOpType.add

    with TileContext(nc) as tc:
        with (
            tc.tile_pool(name="const", bufs=1) as cpool,
            tc.tile_pool(name="x2p", bufs=2) as x2pool,
            tc.tile_pool(name="xtp", bufs=2) as xtpool,
            tc.tile_pool(name="tpp", bufs=3) as tppool,
            tc.tile_pool(name="kernp", bufs=2) as kpool,
            tc.tile_pool(name="accp", bufs=4) as apool,
            tc.tile_pool(name="prodp", bufs=3) as ppool,
            tc.tile_pool(name="ps1", bufs=2, space="PSUM") as ps1pool,
            tc.tile_pool(name="ps2", bufs=2, space="PSUM") as ps2pool,
        ):
            w1xt = cpool.tile([C, CW], BF16, tag="w1x")
            w1t = w1xt[:, 0:CR + 1]
            w2t = w1xt[0:CR + 1, CR + 1:CR + 1 + E]
            b1t = w1xt[0:CR + 1, CR + 2 + E:CR + 4 + E].bitcast(F32)
            # activation-table preload: memzero a dummy, run one Relu on it
            dummy = cpool.tile([1, 2], BF16, tag="dummy")
            nc.vector.memzero(dummy[:])
            nc.scalar.activation(dummy[:], dummy[:], relu)
            # PE pstate warmup: keep the PE busy ~3us from t~0.5 so the
            # first real mm1 tiles run at full clock
            warm = cpool.tile([64, MM1_MM], BF16, tag="warm")
            wps = ps1pool.tile([17, MM1_MM], F32, tag="warmps")
            nc.vector.memzero(warm[:])
            for _ in range(4):
                nc.tensor.matmul(wps[:], warm[:, 0:17], warm[:],
                                 start=True, stop=True)

            def emit_inputs(blk):
                st0 = blk * 128
                pieces = XT_PIECES0 if blk == 0 else XT_PIECES1
                x2rows = X2ROWS0 if blk == 0 else X2ROWS1
                xtts = []
                x2t = x2pool.tile([128, F1], BF16, tag="x2")
                for pi, (col0, ncols) in enumerate(pieces):
                    lead = CW if blk == 0 and pi == 0 else 0
                    xtt = xtpool.tile([C, lead + ncols], BF16, tag="xt")
                    base = CW + blk * NPIX_BLK + col0 - lead
                    nc.sync.dma_start(
                        xtt[:], xt_d[:, base:base + lead + ncols])
                    xtts.append(xtt[:, lead:lead + ncols])
                    if lead:
                        nc.vector.tensor_copy(w1xt[:], xtt[:, 0:CW])
                    if pi in x2rows:
                        r_lo, r_hi = x2rows[pi]
                        nc.sync.dma_start(
                            x2t[:, r_lo * S1 * C:r_hi * S1 * C],
                            x2_d[st0:st0 + 128,
                                 r_lo * S1 * C:r_hi * S1 * C])
                return x2t, xtts

            def emit_mm(blk, xtts):
                # mm1 (tp = relu(x @ w1a + b1a)) interleaved with
                # demand-driven mm2 (kern[st, q, e] = tp_q @ [w2; b2]).
                # tp row 16 carries the b2 path: w1a col 16 is zero and
                # b1a[16] = 1, so mm1+relu leaves tp[16, :] == 1 and w2b
                # row 16 adds b2.
                pieces = XT_PIECES0 if blk == 0 else XT_PIECES1
                desc = blk == 1  # block 1 streams/consumes high q first
                kern = kpool.tile([128, NQ * E], BF16, tag="kern")
                pend = list(range(0, NQ, KCH))
                if desc:
                    pend.reverse()
                cov_lo, cov_hi = (NPIX_BLK, NPIX_BLK) if desc else (0, 0)
                for pi, (col0, ncols) in enumerate(pieces):
                    xtt = xtts[pi]
                    tpt = tppool.tile([CR + 1, ncols], BF16, tag="tp")
                    trange = list(range(0, ncols, MM1_PS))
                    if desc:
                        trange.reverse()
                    for ti, t0 in enumerate(trange):
                        pcols = min(MM1_PS, ncols - t0)
                        ps = ps1pool.tile([CR + 1, MM1_PS], F32, tag="ps1")
                        for k in range(0, pcols, MM1_MM):
                            mcols = min(MM1_MM, pcols - k)
                            nc.tensor.matmul(
                                ps[:, k:k + mcols],
                                w1t,
                                xtt[:, t0 + k:t0 + k + mcols],
                                start=True, stop=True)
                        nc.scalar.activation(
                            tpt[:, t0:t0 + pcols],
                            ps[:, 0:pcols], relu, bias=b1t)
                        if desc:
                            cov_lo = col0 + t0
                        else:
                            cov_hi = col0 + t0 + pcols
                        while pend:
                            k0 = pend[0]
                            kch = min(KCH, NQ - k0)
                            if k0 * 128 < cov_lo or (k0 + kch) * 128 > cov_hi:
                                break
                            ps2 = ps2pool.tile([128, KCH * E], F32,
                                               tag="ps2")
                            for qq in range(kch):
                                qc = (k0 + qq) * 128 - col0
                                nc.tensor.matmul(
                                    ps2[:, qq * E:(qq + 1) * E],
                                    tpt[:, qc:qc + 128],
                                    w2t,
                                    start=True, stop=True)
                            nc.scalar.copy(
                                kern[:, k0 * E:(k0 + kch) * E],
                                ps2[:, 0:kch * E])
                            pend.pop(0)
                return kern

            def emit_taps(blk, x2t, kern, pre_last=None):
                # involution taps: acc[st, q, c], DVE c<cd | Pool c>=cd
                st0 = blk * 128
                chunks = CHUNKS0 if blk == 0 else CHUNKS1
                x2q = x2t[:].rearrange("p (h w c) -> p h w c", h=S1, w=S1)
                kv = kern[:].rearrange("p (q e) -> p q e", e=E)
                for ci, (q0, qch, cd) in enumerate(chunks):
                    if pre_last is not None and ci == len(chunks) - 1:
                        pre_last()
                    acc = apool.tile([128, qch * C], BF16, tag="acc")
                    prod = ppool.tile([128, qch * C], BF16, tag="prod")
                    av = acc[:].rearrange("p (q c) -> p q c", c=C)
                    pv = prod[:].rearrange("p (q c) -> p q c", c=C)
                    nrow = qch // S  # qh rows in this chunk
                    r0 = q0 // S     # first subtile row of this chunk
                    for p in range(9):
                        di, dj = p // 3, p % 3
                        ks = kv[:, q0:q0 + qch, 4 * p:4 * p + 4]
                        for eng, c0, c1 in (
                            (nc.vector, 0, cd),
                            (nc.gpsimd, cd, C),
                        ):
                            ncg = (c1 - c0) // 4
                            xop = x2q[:, r0 + di:r0 + di + nrow,
                                      dj:dj + S, c0:c1]
                            kop = ks.unsqueeze(2).broadcast_to(
                                [128, qch, ncg, 4])
                            if p == 0:
                                eng.tensor_tensor(
                                    av[:, :, c0:c1], xop, kop, mult)
                            else:
                                eng.tensor_tensor(
                                    pv[:, :, c0:c1], xop, kop, mult)
                                eng.tensor_tensor(
                                    av[:, :, c0:c1], av[:, :, c0:c1],
                                    pv[:, :, c0:c1], add)
                    nc.sync.dma_start(
                        o_d[st0:st0 + 128, q0 * C:(q0 + qch) * C], acc[:])

            x2t0, xtts0 = emit_inputs(0)
            kern0 = emit_mm(0, xtts0)
            state = {}

            def prefetch_b1():
                state["in1"] = emit_inputs(1)

            # block 1's input DMAs are issued before block 0's last tap
            # chunk so SP's in-order queue doesn't park them behind the
            # final out-DMA (which waits for the last tap)
            emit_taps(0, x2t0, kern0, pre_last=prefetch_b1)
            x2t1, xtts1 = state["in1"]
            kern1 = emit_mm(1, xtts1)
            emit_taps(1, x2t1, kern1)
    nc.compile()
    _CACHE["nc"] = nc
    return nc


def _host_prep(x, w1, b1, gamma, beta, mean, var, w2, b2):
    """Per-core input maps. x: [8,192,192,64] f32."""
    a = (gamma / np.sqrt(var + BN_EPS)).astype(NPF32)
    w1a = np.zeros((C, CR + 1), dtype=NPF32)
    w1a[:, :CR] = w1 * a[None, :]          # col 16 stays 0 -> b2 path
    b1a = np.empty((CR + 1,), dtype=NPF32)
    b1a[:CR] = b1 * a + (beta - mean * a)
    b1a[CR] = 1.0                          # relu(0*x + 1) == 1
    w2b = np.concatenate([w2, b2[None, :]], 0).astype(NPBF16)  # [17, 36]
    # pack [w1a | w2b | b1a-f32-bit-pairs] into one [64, 55] bf16 tensor
    w1x = np.zeros((C, CR + E + 4), dtype=NPBF16)
    w1x[:, :CR + 1] = w1a.astype(NPBF16)
    w1x[:CR + 1, CR + 1:CR + 1 + E] = w2b
    w1x[:CR + 1, CR + 2 + E:] = b1a.view(NPBF16).reshape(CR + 1, 2)

    xb = x.astype(NPBF16)
    in_maps = []
    for b in range(B):
        xi = xb[b]
        # x2: per-subtile 14x14x64 1-ring windows
        xp1 = np.zeros((H + 2, W + 2, C), dtype=NPBF16)
        xp1[1:-1, 1:-1] = xi
        s = xp1.strides
        win1 = np.lib.stride_tricks.as_strided(
            xp1, (NG, NG, S1, S1, C), (s[0] * S, s[1] * S, s[0], s[1], s[2]))
        x2 = np.ascontiguousarray(win1).reshape(NST, F1)
        # xt: [64, blk*18432 + q*128 + st_local], q = qh*12+qw interior
        xr = xi.reshape(NB, 8, S, NG, S, C)  # [blk, si_l, qh, sj, qw, c]
        xt = np.ascontiguousarray(
            xr.transpose(5, 0, 2, 4, 1, 3)    # [c, blk, qh, qw, si_l, sj]
        ).reshape(C, NB * NPIX_BLK)
        xt = np.concatenate([w1x, xt], axis=1)  # consts ride up front
        in_maps.append({"x2": x2, "xt": xt})
    return in_maps


def kernel(x, w1, b1, gamma, beta, mean, var, w2, b2, _bench=None):
    nc = _build_program()
    in_maps = _host_prep(np.asarray(x), np.asarray(w1), np.asarray(b1),
                         np.asarray(gamma), np.asarray(beta), np.asarray(mean),
                         np.asarray(var), np.asarray(w2), np.asarray(b2))
    kw = dict(_bench) if _bench else {}
    res = run_bass_kernel_spmd(nc, in_maps, core_ids=list(range(B)), **kw)
    if _bench is not None:
        _bench["result"] = res
    out = np.empty((B, H, W, C), dtype=NPF32)
    for b in range(B):
        ob = res.results[b]["o"].reshape(NG, NG, S, S, C).astype(NPF32)
        out[b] = ob.transpose(0, 2, 1, 3, 4).reshape(H, W, C)
    return out
